# revision 16
# baseline (speedup 1.0000x reference)
"""Trainium2 Bass kernel for nn_BatchedFCN (batched ensemble MLP + max).

Reference computation (per network n of 1024, batch B=256):
    h = relu(x @ W1_n^T + b1); h = relu(h @ W2_n^T + b2); h = relu(h @ W3_n^T + b3)
    h = relu(h @ W4_n^T + b4); y_n = h @ W5_n^T + b5          # [B, 1]
    out[b] = max_n y_n[b]                                      # [B]

Sharding: the 1024 networks are split across 8 NeuronCores (128 nets/core).
Each core computes a partial max over its networks; the host folds the 8
partial results.

Dataflow per core: activations live transposed [features, batch].

L1 runs in fp8(e4m3) DoubleRow mode with full error compensation: x and W1
are each split into exact hi+lo e4m3 pairs (x = xh+xl, W = wh+wl at
power-of-2 scales), and the product is computed as xh*(wh+wl) + xl*wh --
dropping only the xl*wl term (~bf16 rounding level). DoubleRow contracts
256 k-values per instruction at 0.5 cycles/row, so the K=501 contraction
takes 6 matmuls x 128 cycles per net instead of 4 x 256 in bf16.

L2-L4 are bf16 (any fp8 quantization of mid-layer activations measured
>3e-2 end-to-end error -- over budget). Biases enter via an augmented
ones-row: each weight tile has an extra input row carrying the bias and an
extra output column regenerating the ones row for the next layer (L1's bias
uses the ACT engine bias operand; its ones row comes from a constant
injected in the fp8 x tiles).

L5 packs a full network pair into ONE bf16 matmul: lhsT [128, 2] holds net
A's w5 in rows 0:51 and net B's in rows 64:115 (matching the packed h4
layout), yielding [2, 256] per pair. Pairs land in PSUM at partition slots
32*(p%4) x free halves (8 pairs/bank); a DVE max folds each full bank into
the running max (stale garbage rows fold idempotently).
"""

import sys

import numpy as np

try:
    import concourse  # noqa: F401
except ImportError:  # fall back to the container's staged repo
    sys.path.insert(0, "/opt/trn_rl_repo")

import ml_dtypes  # noqa: E402

import concourse.mybir as mybir  # noqa: E402
import concourse.tile as tile  # noqa: E402
from concourse import bacc, bass_utils  # noqa: E402

# Problem shapes (hardcoded per contract)
NN = 1024  # total networks
B = 256  # batch
NCORES = 8
NPC = NN // NCORES  # networks per core = 128
PAIRS = NPC // 2  # 64
GROUPS = 4  # weight-DMA groups per core
GNETS = NPC // GROUPS  # 32 nets per group
GPAIRS = GNETS // 2  # 16 pairs per group

MA = 101  # augmented hidden width (100 + ones row)
M4 = 51  # augmented layer-4 output (50 + ones col)
MP = 128  # L1 output padded to full PE width (DoubleRow needs M in 32/64/128)

SX = 16.0  # fp8 storage scale for x
SW = 1024.0  # fp8 storage scale for W1
SINV = 1.0 / (SX * SW)  # 2^-14, applied by act1
CONST = 128.0  # ones-generator constant: 128*128 = SX*SW

# w1 SBUF layout per net: 4 blocks (hi-c0, hi-c1, lo-c0, lo-c1), each
# [2 subtiles x 128 cols] fp8 = 256 B -> 1024 B per net per partition
W1B = 4 * 2 * MP  # 1024

BF16 = ml_dtypes.bfloat16
E4M3 = ml_dtypes.float8_e4m3

_PROGRAM_CACHE = {}


def _build_program(reps=1):
    """Build the SPMD Bass program (same program for all 8 cores)."""
    nc = bacc.Bacc("TRN2", debug=False, num_devices=NCORES)
    bf = mybir.dt.bfloat16
    f8 = mybir.dt.float8e4
    f32 = mybir.dt.float32
    DR = mybir.MatmulPerfMode.DoubleRow

    xq_d = nc.dram_tensor("xq", [128, 4 * 512], f8, kind="ExternalInput").ap()
    w1_d = nc.dram_tensor("w1p", [128, NPC * W1B], f8, kind="ExternalInput").ap()
    w2_d = nc.dram_tensor("w2p", [MA, NPC * MA], bf, kind="ExternalInput").ap()
    w3_d = nc.dram_tensor("w3p", [MA, NPC * MA], bf, kind="ExternalInput").ap()
    w4_d = nc.dram_tensor("w4p", [MA, NPC * M4], bf, kind="ExternalInput").ap()
    w5_d = nc.dram_tensor("w5p", [128, NPC], bf, kind="ExternalInput").ap()
    out_d = nc.dram_tensor("out", [128, B], f32, kind="ExternalOutput").ap()

    relu = mybir.ActivationFunctionType.Relu

    with tile.TileContext(nc) as tc:
        from contextlib import ExitStack

        with ExitStack() as ctx:
            consts = ctx.enter_context(tc.tile_pool(name="consts", bufs=1))
            wp1 = ctx.enter_context(tc.tile_pool(name="wp1", bufs=2))
            wp2 = ctx.enter_context(tc.tile_pool(name="wp2", bufs=2))
            wp3 = ctx.enter_context(tc.tile_pool(name="wp3", bufs=2))
            wp4 = ctx.enter_context(tc.tile_pool(name="wp4", bufs=2))
            hp = ctx.enter_context(tc.tile_pool(name="hp", bufs=4))
            pp1 = ctx.enter_context(tc.tile_pool(name="pp1", bufs=2, space="PSUM"))
            pp2 = ctx.enter_context(tc.tile_pool(name="pp2", bufs=2, space="PSUM"))
            pp3 = ctx.enter_context(tc.tile_pool(name="pp3", bufs=1, space="PSUM"))
            pp4a = ctx.enter_context(tc.tile_pool(name="pp4a", bufs=1, space="PSUM"))
            pp4b = ctx.enter_context(tc.tile_pool(name="pp4b", bufs=1, space="PSUM"))
            pp5 = ctx.enter_context(tc.tile_pool(name="pp5", bufs=1, space="PSUM"))

            # small constant tensors go on the ACT HWDGE ring so they don't
            # queue behind the first w1 group on the SP ring
            xq = consts.tile([128, 4 * 512], f8)
            nc.scalar.dma_start(xq[:, 0:1024], xq_d[:, 0:1024])
            nc.scalar.dma_start(xq[:, 1024:2048], xq_d[:, 1024:2048])
            w5t = consts.tile([128, NPC], bf)
            nc.scalar.dma_start(w5t, w5_d)
            acc = consts.tile([128, 512], f32)
            nc.vector.memset(acc, -3.0e38)
            # PE p-state warmup: dummy matmuls accumulating -inf into the
            # pair-0 L5 slot while the first weights stream in, folded into
            # acc as a numeric no-op (max(acc, -inf) = acc) so the group is
            # observable and survives DCE. Operand memsets come first so the
            # warmup starts as early as possible.
            p5 = pp5.tile([128, 512], f32)
            dza = consts.tile([1, 64], bf)
            nc.vector.memset(dza, -1.0e19)
            dzb = consts.tile([1, 64], bf)
            nc.vector.memset(dzb, 1.0e19)
            NWARM = 60
            for i in range(NWARM):
                nc.tensor.matmul(
                    p5[0:2, 0:64],
                    lhsT=dza[0:1, 0:2],
                    rhs=dzb,
                    start=(i == 0),
                    stop=(i == NWARM - 1),
                )
            nc.vector.tensor_max(acc[0:2, 0:64], acc[0:2, 0:64], p5[0:2, 0:64])
            # persistent PSUM tiles: relu4 / the L5 fold read rows that the
            # matmuls never write, so these are seeded once and never rotated
            nc.vector.memset(p5, -3.0e38)
            p4a = pp4a.tile([128, 256], f32)
            nc.vector.memset(p4a, 0.0)
            p4b = pp4b.tile([128, 256], f32)
            nc.vector.memset(p4b, 0.0)
            # one-time ACT table load while the first weight DMA is in flight
            warm = consts.tile([1, 2], f32)
            nc.vector.memset(warm, 0.0)
            nc.scalar.activation(warm[0:1, 1:2], warm[0:1, 0:1], relu)

            # rhs views for the 6-product L1 sequence: (block, x-block)
            # blocks: 0=hi-c0 1=hi-c1 2=lo-c0 3=lo-c1 (chunk c: k in
            # [256c, 256c+256), subtile s holds k = 256c + 128s + p)
            def xview(bi):
                return xq[:, bi * 512 : (bi + 1) * 512].rearrange(
                    "p (s n) -> p s n", s=2
                )

            x_hi = (xview(0), xview(1))
            x_lo = (xview(2), xview(3))

            # L1 product schedule per net: (w-block, rhs) with w-blocks
            # hi-c0/hi-c1/lo-c0/lo-c1 at offsets 0/256/512/768
            L1SEQ = (
                (0, x_hi[0]),  # hi . xhi c0
                (1, x_hi[1]),  # hi . xhi c1
                (2, x_hi[0]),  # lo . xhi c0
                (3, x_hi[1]),  # lo . xhi c1
                (0, x_lo[0]),  # hi . xlo c0
                (1, x_lo[1]),  # hi . xlo c1
            )

            # Software pipeline over pairs, stage skews as in the baseline:
            #   L1@0  act1@1  L2,relu2@2  L3,act3@3  L4,relu4@4  L5,fold@5
            SKEW_MAX = 5
            DMA_LEAD = 12
            for _rep in range(reps):
              group_tiles = {}
              p1_t, p2_t, p3_t, p4_t = {}, {}, {}, {}
              h1_t, h2_t, h3_t, h4_t = {}, {}, {}, {}
              for t in range(PAIRS + SKEW_MAX):
                # weight DMAs, prefetched DMA_LEAD steps ahead of first use
                tl = t + DMA_LEAD
                if t == 0 or (tl % GPAIRS == 0 and tl // GPAIRS < GROUPS):
                    g = 0 if t == 0 else tl // GPAIRS
                    w1t = wp1.tile([128, GNETS * W1B], f8, tag="w1")

                    def w1_chunk(n0, n1, g=g, w1t=w1t):
                        nc.sync.dma_start(
                            w1t[:, n0 * W1B : n1 * W1B],
                            w1_d[:, (g * GNETS + n0) * W1B : (g * GNETS + n1) * W1B],
                        )

                    w2t = wp2.tile([MA, GNETS * MA], bf, tag="w2")
                    w3t = wp3.tile([MA, GNETS * MA], bf, tag="w3")
                    w4t = wp4.tile([MA, GNETS * M4], bf, tag="w4")

                    def mid_chunk(dst, src, w, n0, n1, g=g):
                        nc.sync.dma_start(
                            dst[:, n0 * w : n1 * w],
                            src[:, (g * GNETS + n0) * w : (g * GNETS + n1) * w],
                        )

                    if g == 0:
                        # just-in-time order: pair p's L1 chunk must land by
                        # step p while w2/w3/w4 for the first pairs arrive in
                        # time for their (deeper) pipeline stages
                        w1_chunk(0, 1)
                        w1_chunk(1, 2)
                        w1_chunk(2, 4)
                        mid_chunk(w2t, w2_d, MA, 0, 8)
                        mid_chunk(w3t, w3_d, MA, 0, 8)
                        w1_chunk(4, 6)
                        mid_chunk(w4t, w4_d, M4, 0, 8)
                        w1_chunk(6, 8)
                        w1_chunk(8, 12)
                        mid_chunk(w2t, w2_d, MA, 8, 32)
                        w1_chunk(12, 16)
                        mid_chunk(w3t, w3_d, MA, 8, 32)
                        w1_chunk(16, 20)
                        mid_chunk(w4t, w4_d, M4, 8, 32)
                        w1_chunk(20, 24)
                        w1_chunk(24, 32)
                    else:
                        w1_chunk(0, 4)
                        mid_chunk(w2t, w2_d, MA, 0, 32)
                        w1_chunk(4, 12)
                        mid_chunk(w3t, w3_d, MA, 0, 32)
                        w1_chunk(12, 22)
                        mid_chunk(w4t, w4_d, M4, 0, 32)
                        w1_chunk(22, 32)
                    group_tiles[g] = (w1t, w2t, w3t, w4t)

                def loc(p):
                    jj = p % GPAIRS
                    return p // GPAIRS, 2 * jj, 2 * jj + 1

                # ---- PE stage L2 (pair t-2)
                p_ = t - 2
                if 0 <= p_ < PAIRS:
                    g, nA, nB_ = loc(p_)
                    w2t = group_tiles[g][1]
                    h1 = h1_t.pop(p_)
                    p2 = pp2.tile([128, 512], f32, tag="p2")
                    for nl, fo in ((nA, 0), (nB_, B)):
                        nc.tensor.matmul(
                            p2[0:MA, fo : fo + B],
                            lhsT=w2t[:, nl * MA : (nl + 1) * MA],
                            rhs=h1[:, fo : fo + B],
                        )
                    p2_t[p_] = p2

                # ---- PE stage L3 (pair t-3)
                p_ = t - 3
                if 0 <= p_ < PAIRS:
                    g, nA, nB_ = loc(p_)
                    w3t = group_tiles[g][2]
                    h2 = h2_t.pop(p_)
                    p3 = pp3.tile([128, 512], f32, tag="p3")
                    for nl, fo in ((nA, 0), (nB_, B)):
                        nc.tensor.matmul(
                            p3[0:MA, fo : fo + B],
                            lhsT=w3t[:, nl * MA : (nl + 1) * MA],
                            rhs=h2[:, fo : fo + B],
                        )
                    p3_t[p_] = p3

                # ---- PE stage L4 (pair t-4): A at psum rows 0:51, B at 64:115
                p_ = t - 4
                if 0 <= p_ < PAIRS:
                    g, nA, nB_ = loc(p_)
                    w4t = group_tiles[g][3]
                    h3 = h3_t.pop(p_)
                    p4 = p4a if p_ % 2 == 0 else p4b
                    nc.tensor.matmul(
                        p4[0:M4, 0:B],
                        lhsT=w4t[:, nA * M4 : (nA + 1) * M4],
                        rhs=h3[:, 0:B],
                    )
                    nc.tensor.matmul(
                        p4[64 : 64 + M4, 0:B],
                        lhsT=w4t[:, nB_ * M4 : (nB_ + 1) * M4],
                        rhs=h3[:, B : 2 * B],
                    )
                    p4_t[p_] = p4

                # ---- PE stage L5 (pair t-5): one matmul per pair; pair p ->
                # psum slot (32*(p%4) partitions, (p//4)%2 free half)
                p_ = t - 5
                if 0 <= p_ < PAIRS:
                    h4 = h4_t.pop(p_)
                    pos = p_ % 4
                    fo5 = ((p_ // 4) % 2) * B
                    nc.tensor.matmul(
                        p5[32 * pos : 32 * pos + 2, fo5 : fo5 + B],
                        lhsT=w5t[:, 2 * p_ : 2 * p_ + 2],
                        rhs=h4,
                        tile_position=(0, 32 * pos),
                    )
                    if p_ % 8 == 7 or p_ == PAIRS - 1:
                        nc.vector.tensor_max(acc, acc, p5)

                # ---- PE stage L1 (pair t): 6 DoubleRow products per net
                p_ = t
                if 0 <= p_ < PAIRS:
                    g, nA, nB_ = loc(p_)
                    w1t = group_tiles[g][0]
                    p1 = pp1.tile([128, 512], f32, tag="p1")
                    for nl, fo in ((nA, 0), (nB_, B)):
                        base = nl * W1B
                        for i, (wb, xv) in enumerate(L1SEQ):
                            lhsT = w1t[
                                :, base + wb * 256 : base + (wb + 1) * 256
                            ].rearrange("p (s m) -> p s m", s=2)
                            nc.tensor.matmul(
                                p1[0:MP, fo : fo + B],
                                lhsT=lhsT,
                                rhs=xv,
                                perf_mode=DR,
                                start=(i == 0),
                                stop=(i == 5),
                            )
                    p1_t[p_] = p1

                # ---- ACT stage act1 (pair t-1): relu + descale (b1 is
                # folded into two fp8 constant k-rows of the L1 matmul)
                p_ = t - 1
                if 0 <= p_ < PAIRS:
                    p1 = p1_t.pop(p_)
                    h1 = hp.tile([MA, 512], bf, tag="h1")
                    nc.scalar.activation(h1, p1[0:MA, :], relu, scale=SINV)
                    h1_t[p_] = h1

                # ---- ACT stage act3 (pair t-3, same step as L3)
                p_ = t - 3
                if 0 <= p_ < PAIRS:
                    p3 = p3_t.pop(p_)
                    h3 = hp.tile([MA, 512], bf, tag="h3")
                    nc.scalar.activation(h3, p3[0:MA, :], relu)
                    h3_t[p_] = h3

                # ---- DVE stage relu2 (pair t-2, same step as L2)
                p_ = t - 2
                if 0 <= p_ < PAIRS:
                    p2 = p2_t.pop(p_)
                    h2 = hp.tile([MA, 512], bf, tag="h2")
                    nc.vector.tensor_scalar_max(h2, p2[0:MA, :], 0.0)
                    h2_t[p_] = h2

                # ---- DVE stage relu4 (pair t-4): single [128, 256] pass; psum
                # rows 50/114 carry the ones for L5's bias row, rows 51:64 and
                # 115:128 are harmless zeros (pp4 banks are memset once)
                p_ = t - 4
                if 0 <= p_ < PAIRS:
                    p4 = p4_t.pop(p_)
                    h4 = hp.tile([128, B], bf, tag="h4")
                    nc.vector.tensor_scalar_max(h4, p4, 0.0)
                    h4_t[p_] = h4

            # fold free halves and ship the whole accumulator; rows never
            # written by an L5 slot hold -3e38 and vanish in the host max
            nc.vector.tensor_max(acc[:, 0:B], acc[:, 0:B], acc[:, B : 2 * B])
            nc.sync.dma_start(out_d, acc[:, 0:B])

    nc.compile()
    return nc


def _get_program():
    if "nc" not in _PROGRAM_CACHE:
        _PROGRAM_CACHE["nc"] = _build_program()
    return _PROGRAM_CACHE["nc"]


def _pack_inputs(inputs):
    """Host-side: transpose, augment, split hi/lo fp8 for L1, shard."""
    x = np.asarray(inputs["x"], np.float32)
    w = {i: np.asarray(inputs[f"w{i}"], np.float32) for i in (1, 2, 3, 4, 5)}
    b = {i: np.asarray(inputs[f"b{i}"], np.float32) for i in (1, 2, 3, 4, 5)}

    # ---- x: hi/lo e4m3 split at scale SX, chunked for DoubleRow
    xT = x.T  # [500, 256]
    xh = (SX * xT).astype(E4M3)
    xl = (SX * xT - xh.astype(np.float32)).astype(E4M3)
    xq = np.zeros((128, 4 * 512), E4M3)

    def x_block(bi, src, k0):
        # block bi cols [512*bi : 512*bi+512] = [sub0 256b | sub1 256b],
        # subtile s row p holds k = k0 + 128s + p
        for s in (0, 1):
            ks, ke = k0 + 128 * s, min(k0 + 128 * s + 128, 500)
            if ke > ks:
                xq[0 : ke - ks, bi * 512 + s * B : bi * 512 + s * B + B] = src[ks:ke]

    x_block(0, xh, 0)
    x_block(1, xh, 256)
    x_block(2, xl, 0)
    x_block(3, xl, 256)
    # constant rows live in block hi-c1, subtile 1 (k = 384 + p):
    # k=500 (p=116) ones-generator, k=501/502 (p=117/118) bias hi/lo
    xq[116:119, 1 * 512 + 1 * B : 1 * 512 + 1 * B + B] = np.float32(CONST)

    # ---- W1: transpose, hi/lo e4m3 at scale SW, ones-gen col at m=100
    wt = np.ascontiguousarray(w[1].transpose(0, 2, 1))  # [N, 500, 100]
    whi = (SW * wt).astype(E4M3)
    wlo = (SW * wt - whi.astype(np.float32)).astype(E4M3)

    def w1_blocks(src, consts_):
        k = np.zeros((NN, 512, MP), E4M3)
        k[:, :500, :100] = src
        if consts_:
            k[:, 500, 100] = np.float32(CONST)
            # bias rows: contribution = CONST * raw / (SX*SW) = raw / CONST
            bhi = (CONST * b[1]).astype(E4M3)  # [N, 100]
            blo = (CONST * b[1] - bhi.astype(np.float32)).astype(E4M3)
            k[:, 501, :100] = bhi
            k[:, 502, :100] = blo
        # [N, 512, 128m] -> [N, chunk, sub, p, m]
        return k.reshape(NN, 2, 2, 128, MP)

    khi = w1_blocks(whi, True)
    klo = w1_blocks(wlo, False)
    # per-net layout: blocks (hi-c0, hi-c1, lo-c0, lo-c1), block = [s, m]
    w1p = np.concatenate([khi, klo], axis=1)  # [N, 4(bi), 2(s), 128(p), 128(m)]
    w1p = np.ascontiguousarray(w1p.transpose(3, 0, 1, 2, 4))  # [p, N, bi, s, m]

    def aug_mid(wi, bi_):
        # -> [101(part=i), N, 101]; bias row 100 + ones-gen col 100
        A = np.zeros((NN, MA, MA), np.float32)
        A[:, :100, :100] = wi.transpose(0, 2, 1)
        A[:, 100, :100] = bi_
        A[:, 100, 100] = 1.0
        return A.transpose(1, 0, 2).astype(BF16)

    w2p = aug_mid(w[2], b[2])
    w3p = aug_mid(w[3], b[3])
    w4p = np.zeros((NN, MA, M4), np.float32)
    w4p[:, :100, :50] = w[4].transpose(0, 2, 1)
    w4p[:, 100, :50] = b[4]
    w4p[:, 100, 50] = 1.0
    w4p = w4p.transpose(1, 0, 2).astype(BF16)  # [101, N, 51]

    # ---- W5 pair tile: net 2j in rows 0:51 of col 2j, net 2j+1 in 64:115
    w5a = np.zeros((NN, M4), np.float32)
    w5a[:, :50] = w[5][:, 0, :]
    w5a[:, 50] = b[5][:, 0]

    in_maps = []
    for c in range(NCORES):
        sl = slice(c * NPC, (c + 1) * NPC)
        loc5 = w5a[sl]
        w5p = np.zeros((128, NPC), np.float32)
        for j in range(PAIRS):
            w5p[0:M4, 2 * j] = loc5[2 * j]
            w5p[64 : 64 + M4, 2 * j + 1] = loc5[2 * j + 1]
        in_maps.append(
            {
                "xq": xq,
                "w1p": np.ascontiguousarray(w1p[:, sl].reshape(128, NPC * W1B)),
                "w2p": np.ascontiguousarray(w2p[:, sl].reshape(MA, NPC * MA)),
                "w3p": np.ascontiguousarray(w3p[:, sl].reshape(MA, NPC * MA)),
                "w4p": np.ascontiguousarray(w4p[:, sl].reshape(MA, NPC * M4)),
                "w5p": w5p.astype(BF16),
            }
        )
    return in_maps


def _fold_outputs(results):
    r = np.stack([np.asarray(res["out"], np.float32) for res in results])  # [8,128,256]
    return np.ascontiguousarray(r.max(axis=(0, 1)).astype(np.float32))


def run(inputs, **run_kwargs):
    """Pack, execute on 8 cores, fold. Returns (output[B], BassKernelResults)."""
    nc = _get_program()
    in_maps = _pack_inputs(inputs)
    res = bass_utils.run_bass_kernel_spmd(
        nc, in_maps, core_ids=list(range(NCORES)), **run_kwargs
    )
    return _fold_outputs(res.results), res


def kernel(**inputs):
    out, _ = run(inputs)
    return out


# revision 19
# speedup vs baseline: 1.0458x; 1.0458x over previous
"""Trainium2 Bass kernel for nn_BatchedFCN (batched ensemble MLP + max).

Reference computation (per network n of 1024, batch B=256):
    h = relu(x @ W1_n^T + b1); h = relu(h @ W2_n^T + b2); h = relu(h @ W3_n^T + b3)
    h = relu(h @ W4_n^T + b4); y_n = h @ W5_n^T + b5          # [B, 1]
    out[b] = max_n y_n[b]                                      # [B]

Sharding: the 1024 networks are split across 8 NeuronCores (128 nets/core).
Each core computes a partial max over its networks; the host folds the 8
partial results.

Dataflow per core: activations live transposed [features, batch].

L1 runs in fp8(e4m3) DoubleRow mode with full error compensation: x and W1
are each split into exact hi+lo e4m3 pairs (x = xh+xl, W = wh+wl at
power-of-2 scales), and the product is computed as xh*(wh+wl) + xl*wh --
dropping only the xl*wl term (~bf16 rounding level). DoubleRow contracts
256 k-values per instruction at 0.5 cycles/row, so the K=501 contraction
takes 6 matmuls x 128 cycles per net instead of 4 x 256 in bf16.

L2-L4 are bf16 (any fp8 quantization of mid-layer activations measured
>3e-2 end-to-end error -- over budget). Biases enter via an augmented
ones-row: each weight tile has an extra input row carrying the bias and an
extra output column regenerating the ones row for the next layer (L1's bias
uses the ACT engine bias operand; its ones row comes from a constant
injected in the fp8 x tiles).

L5 packs a full network pair into ONE bf16 matmul: lhsT [128, 2] holds net
A's w5 in rows 0:51 and net B's in rows 64:115 (matching the packed h4
layout), yielding [2, 256] per pair. Pairs land in PSUM at partition slots
32*(p%4) x free halves (8 pairs/bank); a DVE max folds each full bank into
the running max (stale garbage rows fold idempotently).
"""

import sys

import numpy as np

try:
    import concourse  # noqa: F401
except ImportError:  # fall back to the container's staged repo
    sys.path.insert(0, "/opt/trn_rl_repo")

import ml_dtypes  # noqa: E402

import concourse.mybir as mybir  # noqa: E402
import concourse.tile as tile  # noqa: E402
from concourse import bacc, bass_utils  # noqa: E402

# Problem shapes (hardcoded per contract)
NN = 1024  # total networks
B = 256  # batch
NCORES = 8
NPC = NN // NCORES  # networks per core = 128
PAIRS = NPC // 2  # 64
GROUPS = 4  # weight-DMA groups per core
GNETS = NPC // GROUPS  # 32 nets per group
GPAIRS = GNETS // 2  # 16 pairs per group

MA = 101  # augmented hidden width (100 + ones row)
M4 = 51  # augmented layer-4 output (50 + ones col)
MP = 128  # L1 output padded to full PE width (DoubleRow needs M in 32/64/128)

SX = 16.0  # fp8 storage scale for x
SW = 1024.0  # fp8 storage scale for W1
SINV = 1.0 / (SX * SW)  # 2^-14, applied by act1
CONST = 128.0  # ones-generator constant: 128*128 = SX*SW

# w1 SBUF layout per net: 4 blocks (hi-c0, hi-c1, lo-c0, lo-c1), each
# [2 subtiles x 128 cols] fp8 = 256 B -> 1024 B per net per partition
W1B = 4 * 2 * MP  # 1024

BF16 = ml_dtypes.bfloat16
E4M3 = ml_dtypes.float8_e4m3

_PROGRAM_CACHE = {}


def _build_program(reps=1):
    """Build the SPMD Bass program (same program for all 8 cores)."""
    nc = bacc.Bacc("TRN2", debug=False, num_devices=NCORES)
    bf = mybir.dt.bfloat16
    f8 = mybir.dt.float8e4
    f32 = mybir.dt.float32
    DR = mybir.MatmulPerfMode.DoubleRow

    xq_d = nc.dram_tensor("xq", [128, 4 * 512], f8, kind="ExternalInput").ap()
    w1_d = nc.dram_tensor("w1p", [128, NPC * W1B], f8, kind="ExternalInput").ap()
    w2_d = nc.dram_tensor("w2p", [MA, NPC * MA], bf, kind="ExternalInput").ap()
    w3_d = nc.dram_tensor("w3p", [MA, NPC * MA], bf, kind="ExternalInput").ap()
    w4_d = nc.dram_tensor("w4p", [MA, NPC * M4], bf, kind="ExternalInput").ap()
    w5_d = nc.dram_tensor("w5p", [128, NPC], bf, kind="ExternalInput").ap()
    out_d = nc.dram_tensor("out", [128, B], f32, kind="ExternalOutput").ap()

    relu = mybir.ActivationFunctionType.Relu

    with tile.TileContext(nc) as tc:
        from contextlib import ExitStack

        with ExitStack() as ctx:
            consts = ctx.enter_context(tc.tile_pool(name="consts", bufs=1))
            wp1 = ctx.enter_context(tc.tile_pool(name="wp1", bufs=2))
            wp2 = ctx.enter_context(tc.tile_pool(name="wp2", bufs=2))
            wp3 = ctx.enter_context(tc.tile_pool(name="wp3", bufs=2))
            wp4 = ctx.enter_context(tc.tile_pool(name="wp4", bufs=2))
            hp = ctx.enter_context(tc.tile_pool(name="hp", bufs=4))
            pp1 = ctx.enter_context(tc.tile_pool(name="pp1", bufs=2, space="PSUM"))
            pp2 = ctx.enter_context(tc.tile_pool(name="pp2", bufs=2, space="PSUM"))
            pp3 = ctx.enter_context(tc.tile_pool(name="pp3", bufs=1, space="PSUM"))
            pp4a = ctx.enter_context(tc.tile_pool(name="pp4a", bufs=1, space="PSUM"))
            pp4b = ctx.enter_context(tc.tile_pool(name="pp4b", bufs=1, space="PSUM"))
            pp5 = ctx.enter_context(tc.tile_pool(name="pp5", bufs=1, space="PSUM"))

            # small constant tensors go on the ACT HWDGE ring so they don't
            # queue behind the first w1 group on the SP ring
            xq = consts.tile([128, 4 * 512], f8)
            nc.scalar.dma_start(xq, xq_d)
            w5t = consts.tile([128, NPC], bf)
            nc.scalar.dma_start(w5t, w5_d)
            acc = consts.tile([128, 512], f32)
            nc.vector.memset(acc, -3.0e38)
            # PE p-state warmup: dummy matmuls accumulating -inf into the
            # pair-0 L5 slot while the first weights stream in, folded into
            # acc as a numeric no-op (max(acc, -inf) = acc) so the group is
            # observable and survives DCE. Operand memsets come first so the
            # warmup starts as early as possible.
            p5 = pp5.tile([128, 512], f32)
            dza = consts.tile([1, 64], bf)
            nc.vector.memset(dza, -1.0e19)
            dzb = consts.tile([1, 64], bf)
            nc.vector.memset(dzb, 1.0e19)
            NWARM = 30
            for i in range(NWARM):
                nc.tensor.matmul(
                    p5[0:2, 0:64],
                    lhsT=dza[0:1, 0:2],
                    rhs=dzb,
                    start=(i == 0),
                    stop=(i == NWARM - 1),
                )
            nc.vector.tensor_max(acc[0:2, 0:64], acc[0:2, 0:64], p5[0:2, 0:64])
            # persistent PSUM tiles: relu4 / the L5 fold read rows that the
            # matmuls never write, so these are seeded once and never rotated
            nc.vector.memset(p5, -3.0e38)
            p4a = pp4a.tile([128, 256], f32)
            nc.vector.memset(p4a, 0.0)
            p4b = pp4b.tile([128, 256], f32)
            nc.vector.memset(p4b, 0.0)
            # one-time ACT table load while the first weight DMA is in flight
            warm = consts.tile([1, 2], f32)
            nc.vector.memset(warm, 0.0)
            nc.scalar.activation(warm[0:1, 1:2], warm[0:1, 0:1], relu)

            # rhs views for the 6-product L1 sequence: (block, x-block)
            # blocks: 0=hi-c0 1=hi-c1 2=lo-c0 3=lo-c1 (chunk c: k in
            # [256c, 256c+256), subtile s holds k = 256c + 128s + p)
            def xview(bi):
                return xq[:, bi * 512 : (bi + 1) * 512].rearrange(
                    "p (s n) -> p s n", s=2
                )

            x_hi = (xview(0), xview(1))
            x_lo = (xview(2), xview(3))

            # L1 product schedule per net: (w-block, rhs) with w-blocks
            # hi-c0/hi-c1/lo-c0/lo-c1 at offsets 0/256/512/768
            L1SEQ = (
                (0, x_hi[0]),  # hi . xhi c0
                (1, x_hi[1]),  # hi . xhi c1
                (2, x_hi[0]),  # lo . xhi c0
                (3, x_hi[1]),  # lo . xhi c1
                (0, x_lo[0]),  # hi . xlo c0
                (1, x_lo[1]),  # hi . xlo c1
            )

            # Software pipeline over pairs, stage skews as in the baseline:
            #   L1@0  act1@1  L2,relu2@2  L3,act3@3  L4,relu4@4  L5,fold@5
            SKEW_MAX = 5
            DMA_LEAD = 12
            for _rep in range(reps):
              group_tiles = {}
              p1_t, p2_t, p3_t, p4_t = {}, {}, {}, {}
              h1_t, h2_t, h3_t, h4_t = {}, {}, {}, {}
              for t in range(PAIRS + SKEW_MAX):
                # weight DMAs, prefetched DMA_LEAD steps ahead of first use
                tl = t + DMA_LEAD
                if t == 0 or (tl % GPAIRS == 0 and tl // GPAIRS < GROUPS):
                    g = 0 if t == 0 else tl // GPAIRS
                    w1t = wp1.tile([128, GNETS * W1B], f8, tag="w1")

                    def w1_chunk(n0, n1, g=g, w1t=w1t):
                        nc.sync.dma_start(
                            w1t[:, n0 * W1B : n1 * W1B],
                            w1_d[:, (g * GNETS + n0) * W1B : (g * GNETS + n1) * W1B],
                        )

                    w2t = wp2.tile([MA, GNETS * MA], bf, tag="w2")
                    w3t = wp3.tile([MA, GNETS * MA], bf, tag="w3")
                    w4t = wp4.tile([MA, GNETS * M4], bf, tag="w4")

                    def mid_chunk(dst, src, w, n0, n1, g=g):
                        nc.sync.dma_start(
                            dst[:, n0 * w : n1 * w],
                            src[:, (g * GNETS + n0) * w : (g * GNETS + n1) * w],
                        )

                    # need-ordered stream: pair p's w1 chunk by step p, the
                    # mid-layer weights for pairs [4q, 4q+4) by steps 4q+2..4
                    for q in range(4):
                        w1_chunk(8 * q, 8 * q + 2)
                        if q > 0:
                            w1_chunk(8 * q + 2, 8 * q + 4)
                        mid_chunk(w2t, w2_d, MA, 8 * q, 8 * q + 8)
                        if q == 0:
                            w1_chunk(2, 4)
                        w1_chunk(8 * q + 4, 8 * q + 6)
                        mid_chunk(w3t, w3_d, MA, 8 * q, 8 * q + 8)
                        w1_chunk(8 * q + 6, 8 * q + 8)
                        mid_chunk(w4t, w4_d, M4, 8 * q, 8 * q + 8)
                    group_tiles[g] = (w1t, w2t, w3t, w4t)

                def loc(p):
                    jj = p % GPAIRS
                    return p // GPAIRS, 2 * jj, 2 * jj + 1

                # ---- PE stage L2 (pair t-2)
                p_ = t - 2
                if 0 <= p_ < PAIRS:
                    g, nA, nB_ = loc(p_)
                    w2t = group_tiles[g][1]
                    h1 = h1_t.pop(p_)
                    p2 = pp2.tile([128, 512], f32, tag="p2")
                    for nl, fo in ((nA, 0), (nB_, B)):
                        nc.tensor.matmul(
                            p2[0:MA, fo : fo + B],
                            lhsT=w2t[:, nl * MA : (nl + 1) * MA],
                            rhs=h1[:, fo : fo + B],
                        )
                    p2_t[p_] = p2

                # ---- PE stage L3 (pair t-3)
                p_ = t - 3
                if 0 <= p_ < PAIRS:
                    g, nA, nB_ = loc(p_)
                    w3t = group_tiles[g][2]
                    h2 = h2_t.pop(p_)
                    p3 = pp3.tile([128, 512], f32, tag="p3")
                    for nl, fo in ((nA, 0), (nB_, B)):
                        nc.tensor.matmul(
                            p3[0:MA, fo : fo + B],
                            lhsT=w3t[:, nl * MA : (nl + 1) * MA],
                            rhs=h2[:, fo : fo + B],
                        )
                    p3_t[p_] = p3

                # ---- PE stage L4 (pair t-4): A at psum rows 0:51, B at 64:115
                p_ = t - 4
                if 0 <= p_ < PAIRS:
                    g, nA, nB_ = loc(p_)
                    w4t = group_tiles[g][3]
                    h3 = h3_t.pop(p_)
                    p4 = p4a if p_ % 2 == 0 else p4b
                    nc.tensor.matmul(
                        p4[0:M4, 0:B],
                        lhsT=w4t[:, nA * M4 : (nA + 1) * M4],
                        rhs=h3[:, 0:B],
                    )
                    nc.tensor.matmul(
                        p4[64 : 64 + M4, 0:B],
                        lhsT=w4t[:, nB_ * M4 : (nB_ + 1) * M4],
                        rhs=h3[:, B : 2 * B],
                    )
                    p4_t[p_] = p4

                # ---- PE stage L5 (pair t-5): one matmul per pair; pair p ->
                # psum slot (32*(p%4) partitions, (p//4)%2 free half)
                p_ = t - 5
                if 0 <= p_ < PAIRS:
                    h4 = h4_t.pop(p_)
                    pos = p_ % 4
                    fo5 = ((p_ // 4) % 2) * B
                    nc.tensor.matmul(
                        p5[32 * pos : 32 * pos + 2, fo5 : fo5 + B],
                        lhsT=w5t[:, 2 * p_ : 2 * p_ + 2],
                        rhs=h4,
                        tile_position=(0, 32 * pos),
                    )
                    if p_ % 8 == 7 or p_ == PAIRS - 1:
                        nc.vector.tensor_max(acc, acc, p5)

                # ---- PE stage L1 (pair t): 6 DoubleRow products per net
                p_ = t
                if 0 <= p_ < PAIRS:
                    g, nA, nB_ = loc(p_)
                    w1t = group_tiles[g][0]
                    p1 = pp1.tile([128, 512], f32, tag="p1")
                    for nl, fo in ((nA, 0), (nB_, B)):
                        base = nl * W1B
                        for i, (wb, xv) in enumerate(L1SEQ):
                            lhsT = w1t[
                                :, base + wb * 256 : base + (wb + 1) * 256
                            ].rearrange("p (s m) -> p s m", s=2)
                            nc.tensor.matmul(
                                p1[0:MP, fo : fo + B],
                                lhsT=lhsT,
                                rhs=xv,
                                perf_mode=DR,
                                start=(i == 0),
                                stop=(i == 5),
                            )
                    p1_t[p_] = p1

                # ---- ACT stage act1 (pair t-1): relu + descale (b1 is
                # folded into two fp8 constant k-rows of the L1 matmul)
                p_ = t - 1
                if 0 <= p_ < PAIRS:
                    p1 = p1_t.pop(p_)
                    h1 = hp.tile([MA, 512], bf, tag="h1")
                    nc.scalar.activation(h1, p1[0:MA, :], relu, scale=SINV)
                    h1_t[p_] = h1

                # ---- ACT stage act3 (pair t-3, same step as L3)
                p_ = t - 3
                if 0 <= p_ < PAIRS:
                    p3 = p3_t.pop(p_)
                    h3 = hp.tile([MA, 512], bf, tag="h3")
                    nc.scalar.activation(h3, p3[0:MA, :], relu)
                    h3_t[p_] = h3

                # ---- DVE stage relu2 (pair t-2, same step as L2)
                p_ = t - 2
                if 0 <= p_ < PAIRS:
                    p2 = p2_t.pop(p_)
                    h2 = hp.tile([MA, 512], bf, tag="h2")
                    nc.vector.tensor_scalar_max(h2, p2[0:MA, :], 0.0)
                    h2_t[p_] = h2

                # ---- DVE stage relu4 (pair t-4): single [128, 256] pass; psum
                # rows 50/114 carry the ones for L5's bias row, rows 51:64 and
                # 115:128 are harmless zeros (pp4 banks are memset once)
                p_ = t - 4
                if 0 <= p_ < PAIRS:
                    p4 = p4_t.pop(p_)
                    h4 = hp.tile([128, B], bf, tag="h4")
                    nc.vector.tensor_scalar_max(h4, p4, 0.0)
                    h4_t[p_] = h4

            # fold free halves and ship the whole accumulator; rows never
            # written by an L5 slot hold -3e38 and vanish in the host max
            nc.vector.tensor_max(acc[:, 0:B], acc[:, 0:B], acc[:, B : 2 * B])
            nc.sync.dma_start(out_d, acc[:, 0:B])

    nc.compile()
    return nc


def _get_program():
    if "nc" not in _PROGRAM_CACHE:
        _PROGRAM_CACHE["nc"] = _build_program()
    return _PROGRAM_CACHE["nc"]


def _pack_inputs(inputs):
    """Host-side: transpose, augment, split hi/lo fp8 for L1, shard."""
    x = np.asarray(inputs["x"], np.float32)
    w = {i: np.asarray(inputs[f"w{i}"], np.float32) for i in (1, 2, 3, 4, 5)}
    b = {i: np.asarray(inputs[f"b{i}"], np.float32) for i in (1, 2, 3, 4, 5)}

    # ---- x: hi/lo e4m3 split at scale SX, chunked for DoubleRow
    xT = x.T  # [500, 256]
    xh = (SX * xT).astype(E4M3)
    xl = (SX * xT - xh.astype(np.float32)).astype(E4M3)
    xq = np.zeros((128, 4 * 512), E4M3)

    def x_block(bi, src, k0):
        # block bi cols [512*bi : 512*bi+512] = [sub0 256b | sub1 256b],
        # subtile s row p holds k = k0 + 128s + p
        for s in (0, 1):
            ks, ke = k0 + 128 * s, min(k0 + 128 * s + 128, 500)
            if ke > ks:
                xq[0 : ke - ks, bi * 512 + s * B : bi * 512 + s * B + B] = src[ks:ke]

    x_block(0, xh, 0)
    x_block(1, xh, 256)
    x_block(2, xl, 0)
    x_block(3, xl, 256)
    # constant rows live in block hi-c1, subtile 1 (k = 384 + p):
    # k=500 (p=116) ones-generator, k=501/502 (p=117/118) bias hi/lo
    xq[116:119, 1 * 512 + 1 * B : 1 * 512 + 1 * B + B] = np.float32(CONST)

    # ---- W1: transpose, hi/lo e4m3 at scale SW, ones-gen col at m=100
    wt = np.ascontiguousarray(w[1].transpose(0, 2, 1))  # [N, 500, 100]
    whi = (SW * wt).astype(E4M3)
    wlo = (SW * wt - whi.astype(np.float32)).astype(E4M3)

    def w1_blocks(src, consts_):
        k = np.zeros((NN, 512, MP), E4M3)
        k[:, :500, :100] = src
        if consts_:
            k[:, 500, 100] = np.float32(CONST)
            # bias rows: contribution = CONST * raw / (SX*SW) = raw / CONST
            bhi = (CONST * b[1]).astype(E4M3)  # [N, 100]
            blo = (CONST * b[1] - bhi.astype(np.float32)).astype(E4M3)
            k[:, 501, :100] = bhi
            k[:, 502, :100] = blo
        # [N, 512, 128m] -> [N, chunk, sub, p, m]
        return k.reshape(NN, 2, 2, 128, MP)

    khi = w1_blocks(whi, True)
    klo = w1_blocks(wlo, False)
    # per-net layout: blocks (hi-c0, hi-c1, lo-c0, lo-c1), block = [s, m]
    w1p = np.concatenate([khi, klo], axis=1)  # [N, 4(bi), 2(s), 128(p), 128(m)]
    w1p = np.ascontiguousarray(w1p.transpose(3, 0, 1, 2, 4))  # [p, N, bi, s, m]

    def aug_mid(wi, bi_):
        # -> [101(part=i), N, 101]; bias row 100 + ones-gen col 100
        A = np.zeros((NN, MA, MA), np.float32)
        A[:, :100, :100] = wi.transpose(0, 2, 1)
        A[:, 100, :100] = bi_
        A[:, 100, 100] = 1.0
        return A.transpose(1, 0, 2).astype(BF16)

    w2p = aug_mid(w[2], b[2])
    w3p = aug_mid(w[3], b[3])
    w4p = np.zeros((NN, MA, M4), np.float32)
    w4p[:, :100, :50] = w[4].transpose(0, 2, 1)
    w4p[:, 100, :50] = b[4]
    w4p[:, 100, 50] = 1.0
    w4p = w4p.transpose(1, 0, 2).astype(BF16)  # [101, N, 51]

    # ---- W5 pair tile: net 2j in rows 0:51 of col 2j, net 2j+1 in 64:115
    w5a = np.zeros((NN, M4), np.float32)
    w5a[:, :50] = w[5][:, 0, :]
    w5a[:, 50] = b[5][:, 0]

    in_maps = []
    for c in range(NCORES):
        sl = slice(c * NPC, (c + 1) * NPC)
        loc5 = w5a[sl]
        w5p = np.zeros((128, NPC), np.float32)
        for j in range(PAIRS):
            w5p[0:M4, 2 * j] = loc5[2 * j]
            w5p[64 : 64 + M4, 2 * j + 1] = loc5[2 * j + 1]
        in_maps.append(
            {
                "xq": xq,
                "w1p": np.ascontiguousarray(w1p[:, sl].reshape(128, NPC * W1B)),
                "w2p": np.ascontiguousarray(w2p[:, sl].reshape(MA, NPC * MA)),
                "w3p": np.ascontiguousarray(w3p[:, sl].reshape(MA, NPC * MA)),
                "w4p": np.ascontiguousarray(w4p[:, sl].reshape(MA, NPC * M4)),
                "w5p": w5p.astype(BF16),
            }
        )
    return in_maps


def _fold_outputs(results):
    r = np.stack([np.asarray(res["out"], np.float32) for res in results])  # [8,128,256]
    return np.ascontiguousarray(r.max(axis=(0, 1)).astype(np.float32))


def run(inputs, **run_kwargs):
    """Pack, execute on 8 cores, fold. Returns (output[B], BassKernelResults)."""
    nc = _get_program()
    in_maps = _pack_inputs(inputs)
    res = bass_utils.run_bass_kernel_spmd(
        nc, in_maps, core_ids=list(range(NCORES)), **run_kwargs
    )
    return _fold_outputs(res.results), res


def kernel(**inputs):
    out, _ = run(inputs)
    return out


# revision 21
# speedup vs baseline: 1.0521x; 1.0061x over previous
"""Trainium2 Bass kernel for nn_BatchedFCN (batched ensemble MLP + max).

Reference computation (per network n of 1024, batch B=256):
    h = relu(x @ W1_n^T + b1); h = relu(h @ W2_n^T + b2); h = relu(h @ W3_n^T + b3)
    h = relu(h @ W4_n^T + b4); y_n = h @ W5_n^T + b5          # [B, 1]
    out[b] = max_n y_n[b]                                      # [B]

Sharding: the 1024 networks are split across 8 NeuronCores (128 nets/core).
Each core computes a partial max over its networks; the host folds the 8
partial results.

Dataflow per core: activations live transposed [features, batch].

L1 runs in fp8(e4m3) DoubleRow mode with full error compensation: x and W1
are each split into exact hi+lo e4m3 pairs (x = xh+xl, W = wh+wl at
power-of-2 scales), and the product is computed as xh*(wh+wl) + xl*wh --
dropping only the xl*wl term (~bf16 rounding level). DoubleRow contracts
256 k-values per instruction at 0.5 cycles/row, so the K=501 contraction
takes 6 matmuls x 128 cycles per net instead of 4 x 256 in bf16.

L2-L4 are bf16 (any fp8 quantization of mid-layer activations measured
>3e-2 end-to-end error -- over budget). Biases enter via an augmented
ones-row: each weight tile has an extra input row carrying the bias and an
extra output column regenerating the ones row for the next layer (L1's bias
uses the ACT engine bias operand; its ones row comes from a constant
injected in the fp8 x tiles).

L5 packs a full network pair into ONE bf16 matmul: lhsT [128, 2] holds net
A's w5 in rows 0:51 and net B's in rows 64:115 (matching the packed h4
layout), yielding [2, 256] per pair. Pairs land in PSUM at partition slots
32*(p%4) x free halves (8 pairs/bank); a DVE max folds each full bank into
the running max (stale garbage rows fold idempotently).
"""

import sys

import numpy as np

try:
    import concourse  # noqa: F401
except ImportError:  # fall back to the container's staged repo
    sys.path.insert(0, "/opt/trn_rl_repo")

import ml_dtypes  # noqa: E402

import concourse.mybir as mybir  # noqa: E402
import concourse.tile as tile  # noqa: E402
from concourse import bacc, bass_utils  # noqa: E402

# Problem shapes (hardcoded per contract)
NN = 1024  # total networks
B = 256  # batch
NCORES = 8
NPC = NN // NCORES  # networks per core = 128
PAIRS = NPC // 2  # 64
GROUPS = 4  # weight-DMA groups per core
GNETS = NPC // GROUPS  # 32 nets per group
GPAIRS = GNETS // 2  # 16 pairs per group

MA = 101  # augmented hidden width (100 + ones row)
M4 = 51  # augmented layer-4 output (50 + ones col)
MP = 128  # L1 output padded to full PE width (DoubleRow needs M in 32/64/128)

SX = 16.0  # fp8 storage scale for x
SW = 1024.0  # fp8 storage scale for W1
SINV = 1.0 / (SX * SW)  # 2^-14, applied by act1
CONST = 128.0  # ones-generator constant: 128*128 = SX*SW

# w1 SBUF layout per net: 4 blocks (hi-c0, hi-c1, lo-c0, lo-c1), each
# [2 subtiles x 128 cols] fp8 = 256 B -> 1024 B per net per partition
W1B = 4 * 2 * MP  # 1024

BF16 = ml_dtypes.bfloat16
E4M3 = ml_dtypes.float8_e4m3

_PROGRAM_CACHE = {}


def _build_program(reps=1):
    """Build the SPMD Bass program (same program for all 8 cores)."""
    nc = bacc.Bacc("TRN2", debug=False, num_devices=NCORES)
    bf = mybir.dt.bfloat16
    f8 = mybir.dt.float8e4
    f32 = mybir.dt.float32
    DR = mybir.MatmulPerfMode.DoubleRow

    xq_d = nc.dram_tensor("xq", [128, 4 * 512], f8, kind="ExternalInput").ap()
    w1_d = nc.dram_tensor("w1p", [128, NPC * W1B], f8, kind="ExternalInput").ap()
    w2_d = nc.dram_tensor("w2p", [MA, NPC * MA], bf, kind="ExternalInput").ap()
    w3_d = nc.dram_tensor("w3p", [MA, NPC * MA], bf, kind="ExternalInput").ap()
    w4_d = nc.dram_tensor("w4p", [MA, NPC * M4], bf, kind="ExternalInput").ap()
    w5_d = nc.dram_tensor("w5p", [128, NPC], bf, kind="ExternalInput").ap()
    out_d = nc.dram_tensor("out", [128, 512], f32, kind="ExternalOutput").ap()
    out2_d = nc.dram_tensor("out2", [128, 512], f32, kind="ExternalOutput").ap()

    relu = mybir.ActivationFunctionType.Relu

    with tile.TileContext(nc) as tc:
        from contextlib import ExitStack

        with ExitStack() as ctx:
            consts = ctx.enter_context(tc.tile_pool(name="consts", bufs=1))
            wp1 = ctx.enter_context(tc.tile_pool(name="wp1", bufs=2))
            wp2 = ctx.enter_context(tc.tile_pool(name="wp2", bufs=2))
            wp3 = ctx.enter_context(tc.tile_pool(name="wp3", bufs=2))
            wp4 = ctx.enter_context(tc.tile_pool(name="wp4", bufs=2))
            hp = ctx.enter_context(tc.tile_pool(name="hp", bufs=4))
            pp1 = ctx.enter_context(tc.tile_pool(name="pp1", bufs=2, space="PSUM"))
            pp2 = ctx.enter_context(tc.tile_pool(name="pp2", bufs=2, space="PSUM"))
            pp3 = ctx.enter_context(tc.tile_pool(name="pp3", bufs=1, space="PSUM"))
            pp4a = ctx.enter_context(tc.tile_pool(name="pp4a", bufs=1, space="PSUM"))
            pp4b = ctx.enter_context(tc.tile_pool(name="pp4b", bufs=1, space="PSUM"))
            pp5 = ctx.enter_context(tc.tile_pool(name="pp5", bufs=1, space="PSUM"))

            # small constant tensors go on the ACT HWDGE ring so they don't
            # queue behind the first w1 group on the SP ring
            xq = consts.tile([128, 4 * 512], f8)
            nc.scalar.dma_start(xq, xq_d)
            w5t = consts.tile([128, NPC], bf)
            nc.scalar.dma_start(w5t, w5_d)
            acc = consts.tile([128, 512], f32)
            nc.vector.memset(acc, -3.0e38)
            accB = consts.tile([128, 512], f32)
            # PE p-state warmup: dummy matmuls accumulating -inf into the
            # pair-0 L5 slot while the first weights stream in, folded into
            # acc as a numeric no-op (max(acc, -inf) = acc) so the group is
            # observable and survives DCE. Operand memsets come first so the
            # warmup starts as early as possible.
            p5 = pp5.tile([128, 512], f32)
            dza = consts.tile([1, 64], bf)
            nc.vector.memset(dza, -1.0e19)
            dzb = consts.tile([1, 64], bf)
            nc.vector.memset(dzb, 1.0e19)
            NWARM = 30
            for i in range(NWARM):
                nc.tensor.matmul(
                    p5[0:2, 0:64],
                    lhsT=dza[0:1, 0:2],
                    rhs=dzb,
                    start=(i == 0),
                    stop=(i == NWARM - 1),
                )
            nc.vector.tensor_max(acc[0:2, 0:64], acc[0:2, 0:64], p5[0:2, 0:64])
            # persistent PSUM tiles: relu4 / the L5 fold read rows that the
            # matmuls never write, so these are seeded once and never rotated
            nc.vector.memset(p5, -3.0e38)
            p4a = pp4a.tile([128, 256], f32)
            nc.vector.memset(p4a, 0.0)
            p4b = pp4b.tile([128, 256], f32)
            nc.vector.memset(p4b, 0.0)
            # one-time ACT table load while the first weight DMA is in flight
            warm = consts.tile([1, 2], f32)
            nc.vector.memset(warm, 0.0)
            nc.scalar.activation(warm[0:1, 1:2], warm[0:1, 0:1], relu)

            # rhs views for the 6-product L1 sequence: (block, x-block)
            # blocks: 0=hi-c0 1=hi-c1 2=lo-c0 3=lo-c1 (chunk c: k in
            # [256c, 256c+256), subtile s holds k = 256c + 128s + p)
            def xview(bi):
                return xq[:, bi * 512 : (bi + 1) * 512].rearrange(
                    "p (s n) -> p s n", s=2
                )

            x_hi = (xview(0), xview(1))
            x_lo = (xview(2), xview(3))

            # L1 product schedule per net: (w-block, rhs) with w-blocks
            # hi-c0/hi-c1/lo-c0/lo-c1 at offsets 0/256/512/768
            L1SEQ = (
                (0, x_hi[0]),  # hi . xhi c0
                (1, x_hi[1]),  # hi . xhi c1
                (2, x_hi[0]),  # lo . xhi c0
                (3, x_hi[1]),  # lo . xhi c1
                (0, x_lo[0]),  # hi . xlo c0
                (1, x_lo[1]),  # hi . xlo c1
            )

            # Software pipeline over pairs, stage skews as in the baseline:
            #   L1@0  act1@1  L2,relu2@2  L3,act3@3  L4,relu4@4  L5,fold@5
            SKEW_MAX = 5
            DMA_LEAD = 12
            for _rep in range(reps):
              group_tiles = {}
              p1_t, p2_t, p3_t, p4_t = {}, {}, {}, {}
              h1_t, h2_t, h3_t, h4_t = {}, {}, {}, {}
              for t in range(PAIRS + SKEW_MAX):
                # weight DMAs, prefetched DMA_LEAD steps ahead of first use
                tl = t + DMA_LEAD
                if t == 0 or (tl % GPAIRS == 0 and tl // GPAIRS < GROUPS):
                    g = 0 if t == 0 else tl // GPAIRS
                    w1t = wp1.tile([128, GNETS * W1B], f8, tag="w1")

                    def w1_chunk(n0, n1, g=g, w1t=w1t):
                        nc.sync.dma_start(
                            w1t[:, n0 * W1B : n1 * W1B],
                            w1_d[:, (g * GNETS + n0) * W1B : (g * GNETS + n1) * W1B],
                        )

                    w2t = wp2.tile([MA, GNETS * MA], bf, tag="w2")
                    w3t = wp3.tile([MA, GNETS * MA], bf, tag="w3")
                    w4t = wp4.tile([MA, GNETS * M4], bf, tag="w4")

                    def mid_chunk(dst, src, w, n0, n1, g=g):
                        nc.sync.dma_start(
                            dst[:, n0 * w : n1 * w],
                            src[:, (g * GNETS + n0) * w : (g * GNETS + n1) * w],
                        )

                    # need-ordered stream: pair p's w1 chunk by step p, the
                    # mid-layer weights for pairs [4q, 4q+4) by steps 4q+2..4
                    for q in range(4):
                        w1_chunk(8 * q, 8 * q + 2)
                        if q > 0:
                            w1_chunk(8 * q + 2, 8 * q + 4)
                        mid_chunk(w2t, w2_d, MA, 8 * q, 8 * q + 8)
                        if q == 0:
                            w1_chunk(2, 4)
                        w1_chunk(8 * q + 4, 8 * q + 6)
                        mid_chunk(w3t, w3_d, MA, 8 * q, 8 * q + 8)
                        w1_chunk(8 * q + 6, 8 * q + 8)
                        mid_chunk(w4t, w4_d, M4, 8 * q, 8 * q + 8)
                    group_tiles[g] = (w1t, w2t, w3t, w4t)

                def loc(p):
                    jj = p % GPAIRS
                    return p // GPAIRS, 2 * jj, 2 * jj + 1

                # ---- PE stage L2 (pair t-2)
                p_ = t - 2
                if 0 <= p_ < PAIRS:
                    g, nA, nB_ = loc(p_)
                    w2t = group_tiles[g][1]
                    h1 = h1_t.pop(p_)
                    p2 = pp2.tile([128, 512], f32, tag="p2")
                    for nl, fo in ((nA, 0), (nB_, B)):
                        nc.tensor.matmul(
                            p2[0:MA, fo : fo + B],
                            lhsT=w2t[:, nl * MA : (nl + 1) * MA],
                            rhs=h1[:, fo : fo + B],
                        )
                    p2_t[p_] = p2

                # ---- PE stage L3 (pair t-3)
                p_ = t - 3
                if 0 <= p_ < PAIRS:
                    g, nA, nB_ = loc(p_)
                    w3t = group_tiles[g][2]
                    h2 = h2_t.pop(p_)
                    p3 = pp3.tile([128, 512], f32, tag="p3")
                    for nl, fo in ((nA, 0), (nB_, B)):
                        nc.tensor.matmul(
                            p3[0:MA, fo : fo + B],
                            lhsT=w3t[:, nl * MA : (nl + 1) * MA],
                            rhs=h2[:, fo : fo + B],
                        )
                    p3_t[p_] = p3

                # ---- PE stage L4 (pair t-4): A at psum rows 0:51, B at 64:115
                p_ = t - 4
                if 0 <= p_ < PAIRS:
                    g, nA, nB_ = loc(p_)
                    w4t = group_tiles[g][3]
                    h3 = h3_t.pop(p_)
                    p4 = p4a if p_ % 2 == 0 else p4b
                    nc.tensor.matmul(
                        p4[0:M4, 0:B],
                        lhsT=w4t[:, nA * M4 : (nA + 1) * M4],
                        rhs=h3[:, 0:B],
                    )
                    nc.tensor.matmul(
                        p4[64 : 64 + M4, 0:B],
                        lhsT=w4t[:, nB_ * M4 : (nB_ + 1) * M4],
                        rhs=h3[:, B : 2 * B],
                    )
                    p4_t[p_] = p4

                # ---- PE stage L5 (pair t-5): one matmul per pair; pair p ->
                # psum slot (32*(p%4) partitions, (p//4)%2 free half)
                p_ = t - 5
                if 0 <= p_ < PAIRS:
                    h4 = h4_t.pop(p_)
                    pos = p_ % 4
                    fo5 = ((p_ // 4) % 2) * B
                    nc.tensor.matmul(
                        p5[32 * pos : 32 * pos + 2, fo5 : fo5 + B],
                        lhsT=w5t[:, 2 * p_ : 2 * p_ + 2],
                        rhs=h4,
                        tile_position=(0, 32 * pos),
                    )
                    if p_ % 8 == 7 and p_ != PAIRS - 1:
                        nc.vector.tensor_max(acc, acc, p5)
                        if p_ == PAIRS - 9:
                            # acc is final; ship now so the out-DMA latency
                            # hides behind the last 8 pairs
                            nc.sync.dma_start(out_d, acc)
                    if p_ == PAIRS - 5:
                        # pairs 56-59 (free half 0 of the last bank) are done;
                        # copy out and ship early too
                        nc.vector.tensor_scalar_max(accB[:, 0:B], p5[:, 0:B], -3.0e38)
                        nc.sync.dma_start(out2_d[:, 0:B], accB[:, 0:B])

                # ---- PE stage L1 (pair t): 6 DoubleRow products per net
                p_ = t
                if 0 <= p_ < PAIRS:
                    g, nA, nB_ = loc(p_)
                    w1t = group_tiles[g][0]
                    p1 = pp1.tile([128, 512], f32, tag="p1")
                    for nl, fo in ((nA, 0), (nB_, B)):
                        base = nl * W1B
                        for i, (wb, xv) in enumerate(L1SEQ):
                            lhsT = w1t[
                                :, base + wb * 256 : base + (wb + 1) * 256
                            ].rearrange("p (s m) -> p s m", s=2)
                            nc.tensor.matmul(
                                p1[0:MP, fo : fo + B],
                                lhsT=lhsT,
                                rhs=xv,
                                perf_mode=DR,
                                start=(i == 0),
                                stop=(i == 5),
                            )
                    p1_t[p_] = p1

                # ---- ACT stage act1 (pair t-1): relu + descale (b1 is
                # folded into two fp8 constant k-rows of the L1 matmul)
                p_ = t - 1
                if 0 <= p_ < PAIRS:
                    p1 = p1_t.pop(p_)
                    h1 = hp.tile([MA, 512], bf, tag="h1")
                    nc.scalar.activation(h1, p1[0:MA, :], relu, scale=SINV)
                    h1_t[p_] = h1

                # ---- ACT stage act3 (pair t-3, same step as L3)
                p_ = t - 3
                if 0 <= p_ < PAIRS:
                    p3 = p3_t.pop(p_)
                    h3 = hp.tile([MA, 512], bf, tag="h3")
                    nc.scalar.activation(h3, p3[0:MA, :], relu)
                    h3_t[p_] = h3

                # ---- DVE stage relu2 (pair t-2, same step as L2)
                p_ = t - 2
                if 0 <= p_ < PAIRS:
                    p2 = p2_t.pop(p_)
                    h2 = hp.tile([MA, 512], bf, tag="h2")
                    nc.vector.tensor_scalar_max(h2, p2[0:MA, :], 0.0)
                    h2_t[p_] = h2

                # ---- DVE stage relu4 (pair t-4): single [128, 256] pass; psum
                # rows 50/114 carry the ones for L5's bias row, rows 51:64 and
                # 115:128 are harmless zeros (pp4 banks are memset once)
                p_ = t - 4
                if 0 <= p_ < PAIRS:
                    p4 = p4_t.pop(p_)
                    h4 = hp.tile([128, B], bf, tag="h4")
                    nc.vector.tensor_scalar_max(h4, p4, 0.0)
                    h4_t[p_] = h4

            # pairs 60-63 (free half 1 of the last bank): copy to SBUF and
            # ship; the host max replaces the final fold
            nc.vector.tensor_scalar_max(accB[:, B : 2 * B], p5[:, B : 2 * B], -3.0e38)
            nc.sync.dma_start(out2_d[:, B : 2 * B], accB[:, B : 2 * B])

    nc.compile()
    return nc


def _get_program():
    if "nc" not in _PROGRAM_CACHE:
        _PROGRAM_CACHE["nc"] = _build_program()
    return _PROGRAM_CACHE["nc"]


def _pack_inputs(inputs):
    """Host-side: transpose, augment, split hi/lo fp8 for L1, shard."""
    x = np.asarray(inputs["x"], np.float32)
    w = {i: np.asarray(inputs[f"w{i}"], np.float32) for i in (1, 2, 3, 4, 5)}
    b = {i: np.asarray(inputs[f"b{i}"], np.float32) for i in (1, 2, 3, 4, 5)}

    # ---- x: hi/lo e4m3 split at scale SX, chunked for DoubleRow
    xT = x.T  # [500, 256]
    xh = (SX * xT).astype(E4M3)
    xl = (SX * xT - xh.astype(np.float32)).astype(E4M3)
    xq = np.zeros((128, 4 * 512), E4M3)

    def x_block(bi, src, k0):
        # block bi cols [512*bi : 512*bi+512] = [sub0 256b | sub1 256b],
        # subtile s row p holds k = k0 + 128s + p
        for s in (0, 1):
            ks, ke = k0 + 128 * s, min(k0 + 128 * s + 128, 500)
            if ke > ks:
                xq[0 : ke - ks, bi * 512 + s * B : bi * 512 + s * B + B] = src[ks:ke]

    x_block(0, xh, 0)
    x_block(1, xh, 256)
    x_block(2, xl, 0)
    x_block(3, xl, 256)
    # constant rows live in block hi-c1, subtile 1 (k = 384 + p):
    # k=500 (p=116) ones-generator, k=501/502 (p=117/118) bias hi/lo
    xq[116:119, 1 * 512 + 1 * B : 1 * 512 + 1 * B + B] = np.float32(CONST)

    # ---- W1: transpose, hi/lo e4m3 at scale SW, ones-gen col at m=100
    wt = np.ascontiguousarray(w[1].transpose(0, 2, 1))  # [N, 500, 100]
    whi = (SW * wt).astype(E4M3)
    wlo = (SW * wt - whi.astype(np.float32)).astype(E4M3)

    def w1_blocks(src, consts_):
        k = np.zeros((NN, 512, MP), E4M3)
        k[:, :500, :100] = src
        if consts_:
            k[:, 500, 100] = np.float32(CONST)
            # bias rows: contribution = CONST * raw / (SX*SW) = raw / CONST
            bhi = (CONST * b[1]).astype(E4M3)  # [N, 100]
            blo = (CONST * b[1] - bhi.astype(np.float32)).astype(E4M3)
            k[:, 501, :100] = bhi
            k[:, 502, :100] = blo
        # [N, 512, 128m] -> [N, chunk, sub, p, m]
        return k.reshape(NN, 2, 2, 128, MP)

    khi = w1_blocks(whi, True)
    klo = w1_blocks(wlo, False)
    # per-net layout: blocks (hi-c0, hi-c1, lo-c0, lo-c1), block = [s, m]
    w1p = np.concatenate([khi, klo], axis=1)  # [N, 4(bi), 2(s), 128(p), 128(m)]
    w1p = np.ascontiguousarray(w1p.transpose(3, 0, 1, 2, 4))  # [p, N, bi, s, m]

    def aug_mid(wi, bi_):
        # -> [101(part=i), N, 101]; bias row 100 + ones-gen col 100
        A = np.zeros((NN, MA, MA), np.float32)
        A[:, :100, :100] = wi.transpose(0, 2, 1)
        A[:, 100, :100] = bi_
        A[:, 100, 100] = 1.0
        return A.transpose(1, 0, 2).astype(BF16)

    w2p = aug_mid(w[2], b[2])
    w3p = aug_mid(w[3], b[3])
    w4p = np.zeros((NN, MA, M4), np.float32)
    w4p[:, :100, :50] = w[4].transpose(0, 2, 1)
    w4p[:, 100, :50] = b[4]
    w4p[:, 100, 50] = 1.0
    w4p = w4p.transpose(1, 0, 2).astype(BF16)  # [101, N, 51]

    # ---- W5 pair tile: net 2j in rows 0:51 of col 2j, net 2j+1 in 64:115
    w5a = np.zeros((NN, M4), np.float32)
    w5a[:, :50] = w[5][:, 0, :]
    w5a[:, 50] = b[5][:, 0]

    in_maps = []
    for c in range(NCORES):
        sl = slice(c * NPC, (c + 1) * NPC)
        loc5 = w5a[sl]
        w5p = np.zeros((128, NPC), np.float32)
        for j in range(PAIRS):
            w5p[0:M4, 2 * j] = loc5[2 * j]
            w5p[64 : 64 + M4, 2 * j + 1] = loc5[2 * j + 1]
        in_maps.append(
            {
                "xq": xq,
                "w1p": np.ascontiguousarray(w1p[:, sl].reshape(128, NPC * W1B)),
                "w2p": np.ascontiguousarray(w2p[:, sl].reshape(MA, NPC * MA)),
                "w3p": np.ascontiguousarray(w3p[:, sl].reshape(MA, NPC * MA)),
                "w4p": np.ascontiguousarray(w4p[:, sl].reshape(MA, NPC * M4)),
                "w5p": w5p.astype(BF16),
            }
        )
    return in_maps


def _fold_outputs(results):
    r = np.stack(
        [np.asarray(res[k], np.float32) for res in results for k in ("out", "out2")]
    )  # [16, 128, 512]
    r = r.reshape(16, 128, 2, B)
    return np.ascontiguousarray(r.max(axis=(0, 1, 2)).astype(np.float32))


def run(inputs, **run_kwargs):
    """Pack, execute on 8 cores, fold. Returns (output[B], BassKernelResults)."""
    nc = _get_program()
    in_maps = _pack_inputs(inputs)
    res = bass_utils.run_bass_kernel_spmd(
        nc, in_maps, core_ids=list(range(NCORES)), **run_kwargs
    )
    return _fold_outputs(res.results), res


def kernel(**inputs):
    out, _ = run(inputs)
    return out
